# revision 5
# baseline (speedup 1.0000x reference)
"""Trainium2 Bass kernel: single-head causal attention (v3.0).

Problem: x[4,2048,1024] f32; q/k/v = x@W* + b* (head dim 128);
out = softmax(causal(q k^T / sqrt(128))) @ v.

Sharding: 8 cores = 4 batches x 2 causal "wedges". Within a batch, the 16
query blocks (128 rows each) are interleaved between the two cores
(h=0 takes odd global blocks, h=1 even) so both cores carry an identical
static schedule: slot p attends exactly L_p = 2p+2 local key blocks.
Per-core key order is a host-side permutation (h=0 identity, h=1
adjacent-pair swap) that puts slot p's own (diagonal) block at local
position 2p+1; the wedge difference is carried by a mask input, so a
single NEFF serves all 8 cores (SPMD).

v3 design (from v2.4 trace analysis: DMA-in ends ~27us but 16us of PE
work used to pile up after it; ACT exp chain is the serial mid-phase
resource):
  - All input DMAs issued up-front on the sync queue in priority order
    (consts fp8 -> x8 chunks -> f32/bf16 consts -> xg groups); consts
    packed into 3 tensors to cut issue serialization (~620ns each).
  - q/k projections in fp8 e4m3 with DoubleRow (contraction 256/matmul);
    weights prescaled x8, 1/8 folded into the psum->sbuf copy scale.
  - v projection emits v directly in [key, dk] orientation
    (lhsT = x^T key-block, rhs = Wv chunk): no PE/XBAR transposes at all.
    v bias is NOT applied here: softmax weights sum to 1, so bv is folded
    into the output epilogue (out = o_ps*rcp + bv) as one
    scalar_tensor_tensor.
  - S^T per key block j computed into a 2-bank [128,1024] psum tile
    (bank-aligned matmul pieces), then ONE exp ACTIVATE per block
    (16 total, was 24) -> ACT chain ~10.5us.
  - k/q psum->sbuf affine copies split ACT (kT_lo[0:512], first-needed) /
    DVE (rest) so the first exps start ~15us.
  - PV bursts (slot p over its 2p+2 key blocks, denominator via the
    v_aug ones-column). Bursts 6,7 split: blocks 0..11 accumulate as soon
    as exps land; only blocks 12..15 + finalize remain after xg group 3
    arrives -> ~2.5us tail instead of 16us.
  - All bulk tensors partition-major (>=4KB contiguous runs/partition).
"""

import numpy as np

B, T, D, DK = 4, 2048, 1024, 128
NBLK = T // 128      # 16 key blocks per core
NSLOT = 8            # q slots per core (NSLOT*128 = 1024 q rows)
NCHUNK = D // 128    # bf16 m-chunks (v projection)
NDC = D // 256       # fp8 double-chunks (q/k projections)
NG = 4               # v key groups (512 keys each)
GW = T // NG         # group width (512)
SCALE = 1.0 / np.sqrt(np.float32(DK))
WS = 8.0             # fp8 weight prescale (power of 2; undone in psum copy)
WARMUP_MMS = 5

_built = None


def _build():
    from contextlib import ExitStack

    import concourse.mybir as mybir
    import concourse.tile as tile
    from concourse import bacc

    f32 = mybir.dt.float32
    bf16 = mybir.dt.bfloat16
    fp8 = mybir.dt.float8e4
    Act = mybir.ActivationFunctionType
    Alu = mybir.AluOpType
    DR = mybir.MatmulPerfMode.DoubleRow

    nc = bacc.Bacc("TRN2", target_bir_lowering=False, debug=False, num_devices=8)

    # all bulk inputs partition-major: [128, ...] with >=4KB contiguous runs
    x8p = nc.dram_tensor("x8p", [128, NDC * 2 * T], fp8, kind="ExternalInput").ap()
    xgp = nc.dram_tensor("xgp", [128, NG * NCHUNK * GW], bf16,
                         kind="ExternalInput").ap()
    # packed consts: fp8 = wk8|wq8; f32 = [bq, bk*SCALE, pad, bvv(128)];
    # bf16 = wv | masks
    cstf8 = nc.dram_tensor("cstf8", [128, 2 * NDC * 2 * DK], fp8,
                           kind="ExternalInput").ap()
    cst32 = nc.dram_tensor("cst32", [128, 3 + DK], f32, kind="ExternalInput").ap()
    cst16 = nc.dram_tensor("cst16", [128, NCHUNK * DK + 256], bf16,
                           kind="ExternalInput").ap()
    o = nc.dram_tensor("o", [NSLOT * 128, DK], f32, kind="ExternalOutput").ap()

    with tile.TileContext(nc) as tc, ExitStack() as ctx:
        const = ctx.enter_context(tc.tile_pool(name="const", bufs=1))
        sbufs = ctx.enter_context(tc.tile_pool(name="sbufs", bufs=1))
        x8_pool = ctx.enter_context(tc.tile_pool(name="x8_pool", bufs=NDC))
        xg_pool = ctx.enter_context(tc.tile_pool(name="xg_pool", bufs=NG))
        out_pool = ctx.enter_context(tc.tile_pool(name="out_pool", bufs=3))

        # ---- all input DMAs up-front, priority order
        cstf8_sb = const.tile([128, 2, NDC, 2, DK], fp8, tag="cstf8")
        nc.sync.dma_start(out=cstf8_sb, in_=cstf8)

        x8s = []
        for dc in range(NDC):
            x8t = x8_pool.tile([128, 2, T], fp8, tag="x8", name=f"x8_{dc}")
            x8s.append(x8t)

        def load_x8(dc):
            nc.sync.dma_start(
                out=x8s[dc], in_=x8p[:, 2 * T * dc : 2 * T * (dc + 1)]
            )

        load_x8(0)
        load_x8(1)
        load_x8(2)
        cst32_sb = const.tile([128, 3 + DK], f32, tag="cst32")
        nc.sync.dma_start(out=cst32_sb, in_=cst32)
        load_x8(3)
        cst16_sb = const.tile([128, NCHUNK * DK + 256], bf16, tag="cst16")
        nc.sync.dma_start(out=cst16_sb, in_=cst16)

        xgs = []
        for g in range(NG):
            xg = xg_pool.tile([128, NCHUNK, GW], bf16, tag="xg", name=f"xg{g}")
            xgs.append(xg)
            nc.sync.dma_start(
                out=xg, in_=xgp[:, NCHUNK * GW * g : NCHUNK * GW * (g + 1)]
            )

        wk8_sb = cstf8_sb[:, 0]
        wq8_sb = cstf8_sb[:, 1]
        bq_sb = cst32_sb[:, 0:1]
        bks_sb = cst32_sb[:, 1:2]
        bvv_sb = cst32_sb[:, 3 : 3 + DK]
        wv_sb = cst16_sb[:, 0 : NCHUNK * DK]
        mask_sb = cst16_sb[:, NCHUNK * DK :]

        # v in natural [key, dk] layout, bf16, with a ones column appended
        # (ones column x P gives the softmax denominator for free)
        v_aug = const.tile([128, NBLK, DK + 1], bf16, tag="vaug")
        nc.vector.memset(v_aug[:, :, DK : DK + 1], 1.0)

        # ---- PE warmup: bridge the DMA-wait window at kernel start (PE
        # pstate ramp) and pull the exp ACT_TABLE_LOAD (~1.3us) early.
        with tc.tile_pool(name="warmps", bufs=1, space="PSUM") as warmps:
            wsrc = sbufs.tile([128, 512], bf16, tag="wsrc")
            nc.vector.memset(wsrc, 0.0)
            wdst = warmps.tile([128, 512], f32, tag="warm")
            for _ in range(WARMUP_MMS):
                nc.tensor.matmul(
                    wdst, lhsT=wsrc[:, 0:128], rhs=wsrc, start=True, stop=True
                )
            wexp = sbufs.tile([128, 1], f32, tag="wexp")
            nc.scalar.activation(out=wexp, in_=wsrc[:, 0:1], func=Act.Exp, scale=1.0)

        # ---- q/k projections (fp8 DoubleRow, contraction 256 per matmul).
        # x8 columns host-reordered: own-query blocks (odd locals, slot order)
        # in cols 0:1024, even locals in cols 1024:2048.
        kT_lo = sbufs.tile([128, T // 2], bf16, tag="kTl")  # positions 0..7
        kT_hi = sbufs.tile([128, T // 2], bf16, tag="kTh")  # positions 8..15
        qT_lo = sbufs.tile([128, 512], bf16, tag="qTl")     # slots 0..3
        qT_hi = sbufs.tile([128, 512], bf16, tag="qTh")     # slots 4..7

        kpool = tc.alloc_tile_pool(name="kpool", bufs=1, space="PSUM")
        qpool = tc.alloc_tile_pool(name="qpool", bufs=1, space="PSUM")
        kT_ps = kpool.tile([128, T], f32, tag="kps")
        qT_ps = qpool.tile([128, NSLOT * 128], f32, tag="qps")
        for dc in range(NDC):
            for t in range(4):
                nc.tensor.matmul(
                    kT_ps[:, 512 * t : 512 * (t + 1)],
                    lhsT=wk8_sb[:, dc, :, :],
                    rhs=x8s[dc][:, :, 512 * t : 512 * (t + 1)],
                    start=(dc == 0),
                    stop=(dc == NDC - 1),
                    perf_mode=DR,
                )
            for t in range(2):
                nc.tensor.matmul(
                    qT_ps[:, 512 * t : 512 * (t + 1)],
                    lhsT=wq8_sb[:, dc, :, :],
                    rhs=x8s[dc][:, :, 512 * t : 512 * (t + 1)],
                    start=(dc == 0),
                    stop=(dc == NDC - 1),
                    perf_mode=DR,
                )
        # psum->sbuf affine copies: ACT takes the first-needed kT piece,
        # DVE the rest (ACT must stay free for the exp chain).
        nc.scalar.activation(
            out=kT_lo[:, 0:512], in_=kT_ps[:, 0:512], func=Act.Identity,
            bias=bks_sb, scale=SCALE / WS,
        )
        nc.vector.tensor_scalar(
            out=qT_lo, in0=qT_ps[:, 0:512],
            scalar1=float(1.0 / WS), scalar2=bq_sb, op0=Alu.mult, op1=Alu.add,
        )
        nc.vector.tensor_scalar(
            out=qT_hi, in0=qT_ps[:, 512:1024],
            scalar1=float(1.0 / WS), scalar2=bq_sb, op0=Alu.mult, op1=Alu.add,
        )
        nc.vector.tensor_scalar(
            out=kT_hi[:, 0:512], in0=kT_ps[:, 1024:1536],
            scalar1=float(SCALE / WS), scalar2=bks_sb, op0=Alu.mult, op1=Alu.add,
        )
        nc.vector.tensor_scalar(
            out=kT_hi[:, 512:1024], in0=kT_ps[:, 1536:2048],
            scalar1=float(SCALE / WS), scalar2=bks_sb, op0=Alu.mult, op1=Alu.add,
        )
        nc.vector.tensor_scalar(
            out=kT_lo[:, 512:1024], in0=kT_ps[:, 512:1024],
            scalar1=float(SCALE / WS), scalar2=bks_sb, op0=Alu.mult, op1=Alu.add,
        )
        qpool.release()
        kpool.release()

        # ---- attention: S^T/exp, v groups, PV bursts ----
        pt_pool = ctx.enter_context(tc.tile_pool(name="pt_pool", bufs=NBLK))
        spool = tc.alloc_tile_pool(name="spool", bufs=2, space="PSUM")
        vpool = tc.alloc_tile_pool(name="vpool", bufs=1, space="PSUM")
        opool = tc.alloc_tile_pool(name="opool", bufs=3, space="PSUM")

        pts = [None] * NBLK
        o_pss = [None] * NSLOT

        def kpos(j):
            # column position of local key block j in the reordered x8/kT
            return (j - 1) // 2 if j % 2 == 1 else NSLOT + j // 2

        def emit_st(j):
            """S^T for key block j into a 2-bank psum tile, then one exp
            over the whole active range, then the frontier mask multiply."""
            sj = j // 2           # first active slot for this key position
            q0 = 128 * sj
            qn = NSLOT * 128 - q0
            pt = pt_pool.tile([128, qn], bf16, tag="pt", name=f"pt{j}")
            pts[j] = pt
            kp = kpos(j)
            kt = kT_lo if kp < NSLOT else kT_hi
            kp = kp % NSLOT
            s_ps = spool.tile([128, 1024], f32, tag="st", name=f"s{j}")
            # matmul pieces split at the qT_lo/qT_hi boundary (col 512),
            # which is also the psum bank boundary
            if q0 < 512:
                pieces = [(qT_lo, q0, q0, 512 - q0), (qT_hi, 0, 512, 512)]
            else:
                pieces = [(qT_hi, q0 - 512, q0, 1024 - q0)]
            for qtile, qoff, doff, sz in pieces:
                nc.tensor.matmul(
                    s_ps[:, doff : doff + sz],
                    lhsT=kt[:, 128 * kp : 128 * kp + 128],
                    rhs=qtile[:, qoff : qoff + sz],
                    start=True,
                    stop=True,
                )
            nc.scalar.activation(
                out=pt, in_=s_ps[:, q0:1024], func=Act.Exp, scale=1.0,
            )
            # mask the frontier slot multiplicatively (exp(s+m) = exp(s)*m01):
            # even j -> maskA (wedge-dependent), odd j -> maskB (causal tri)
            sel = j % 2
            nc.vector.tensor_mul(
                pt[:, 0:128],
                pt[:, 0:128],
                mask_sb[:, 128 * sel : 128 * (sel + 1)],
            )

        def emit_vgroup(g):
            """v for key blocks 4g..4g+3, directly in [key, dk] orientation.
            lhsT = x^T key-block (128 keys), rhs = Wv chunk; accumulate over
            the 8 contraction chunks; no bias (folded into the epilogue)."""
            vg_ps = vpool.tile([128, 4, DK], f32, tag="vps", name=f"vg{g}")
            for b in range(4):
                for c in range(NCHUNK):
                    nc.tensor.matmul(
                        vg_ps[:, b, :],
                        lhsT=xgs[g][:, c, 128 * b : 128 * (b + 1)],
                        rhs=wv_sb[:, 128 * c : 128 * (c + 1)],
                        start=(c == 0),
                        stop=(c == NCHUNK - 1),
                    )
            nc.vector.tensor_copy(v_aug[:, 4 * g : 4 * g + 4, 0:DK], vg_ps)

        def emit_burst(p, j_lo, j_hi):
            """PV accumulation for slot p over key blocks j_lo..j_hi-1."""
            if j_lo == 0:
                o_pss[p] = opool.tile([128, DK + 1], f32, tag="o",
                                      name=f"o_ps{p}")
            o_ps = o_pss[p]
            for jj in range(j_lo, j_hi):
                nc.tensor.matmul(
                    o_ps,
                    lhsT=pts[jj][:, 128 * (p - jj // 2) : 128 * (p - jj // 2) + 128],
                    rhs=v_aug[:, jj, :],
                    start=(jj == 0),
                    stop=(jj == 2 * p + 1),
                )

        def emit_finish(p):
            """out = o_ps * (1/den) + bv, then DMA out."""
            o_ps = o_pss[p]
            rcp = out_pool.tile([128, 1], f32, tag="rcp")
            nc.vector.reciprocal(rcp, o_ps[:, DK : DK + 1])
            ob = out_pool.tile([128, DK], f32, tag="ob")
            nc.vector.scalar_tensor_tensor(
                out=ob, in0=o_ps[:, 0:DK], scalar=rcp, in1=bvv_sb,
                op0=Alu.mult, op1=Alu.add,
            )
            nc.sync.dma_start(out=o[128 * p : 128 * (p + 1), :], in_=ob)

        def full_burst(p):
            emit_burst(p, 0, 2 * p + 2)
            emit_finish(p)

        # emission = PE execution order; data-arrival pacing (approx, us):
        #   x8 fully in ~14, xg groups ~17 / 19.5 / 22 / 24.5;
        #   first exp ~15, ACT exp chain paces S^T; PV chases exps.
        emit_st(1); emit_st(0)
        emit_st(3); emit_st(2)
        emit_st(5); emit_st(4)
        emit_st(7); emit_st(6)
        emit_vgroup(0)
        emit_st(9); emit_st(8)
        full_burst(0)
        full_burst(1)
        emit_st(11); emit_st(10)
        emit_vgroup(1)
        full_burst(2)
        emit_st(13); emit_st(12)
        full_burst(3)
        emit_vgroup(2)
        emit_st(15); emit_st(14)
        full_burst(4)
        full_burst(5)
        emit_burst(6, 0, 12)
        emit_burst(7, 0, 12)
        emit_vgroup(3)
        emit_burst(6, 12, 14)
        emit_finish(6)
        emit_burst(7, 12, 16)
        emit_finish(7)

        opool.release()
        vpool.release()
        spool.release()

    nc.compile()
    return nc


def get_built():
    global _built
    if _built is None:
        _built = _build()
    return _built


def _pos2glob(h):
    if h == 0:
        return list(range(NBLK))
    return [j + 1 if j % 2 == 0 else j - 1 for j in range(NBLK)]


def _pack_w_bf16(W):
    """[D, DK] -> [128, NCHUNK*DK] with column block c holding rows 128c..."""
    import ml_dtypes
    return np.ascontiguousarray(
        np.asarray(W, np.float32).reshape(NCHUNK, 128, DK).transpose(1, 0, 2)
        .reshape(128, NCHUNK * DK).astype(ml_dtypes.bfloat16)
    )


def _pack_w_fp8(W):
    """[D, DK] -> [128, NDC*2*DK] e4m3: [p, ((dc*2+i)*DK)+d] = e4m3(WS*W[256dc+128i+p, d])."""
    import ml_dtypes
    Ws = np.asarray(W, np.float32) * WS
    return np.ascontiguousarray(
        Ws.reshape(NDC, 2, 128, DK).transpose(2, 0, 1, 3)
        .reshape(128, NDC * 2 * DK).astype(ml_dtypes.float8_e4m3)
    )


def make_in_map(x_b, Wq, bq, Wk, bk, Wv, bv, h, xT_pre=None, x8T_pre=None):
    """Build one core's input dict. x_b: [T, D] fp32 for this core's batch.
    xT_pre/x8T_pre: optional precomputed transposed/cast copies (shared by
    both wedge cores of a batch; h=0 uses as-is, h=1 column-permutes)."""
    import ml_dtypes
    bf = ml_dtypes.bfloat16
    if xT_pre is None:
        xT_pre = np.ascontiguousarray(x_b.T.astype(bf))
    if x8T_pre is None:
        x8T_pre = np.ascontiguousarray(x_b.T.astype(ml_dtypes.float8_e4m3))
    if h == 0:
        xT_loc, x8T_loc = xT_pre, x8T_pre
    else:
        p2g = _pos2glob(h)
        cols = np.concatenate([np.arange(128 * g, 128 * (g + 1)) for g in p2g])
        xT_loc = np.ascontiguousarray(xT_pre[:, cols])
        x8T_loc = np.ascontiguousarray(x8T_pre[:, cols])
    # x8 column order: own-query blocks (odd locals, slot order) first, then
    # the even locals -- q projection reads cols 0:1024 contiguously
    korder = list(range(1, NBLK, 2)) + list(range(0, NBLK, 2))
    qcols = np.concatenate([np.arange(128 * j, 128 * (j + 1)) for j in korder])
    # x8p[p, (dc*2+i)*T + t] = x8T[256dc+128i+p, perm(t)]  (partition-major)
    x8p = np.ascontiguousarray(
        x8T_loc[:, qcols].reshape(NDC, 2, 128, T).transpose(2, 0, 1, 3)
        .reshape(128, NDC * 2 * T)
    )
    # xgp[p, (g*NCHUNK+c)*GW + t'] = xT[128c+p, GW*g+t']  (partition-major)
    xgp = np.ascontiguousarray(
        xT_loc.reshape(NCHUNK, 128, NG, GW).transpose(1, 2, 0, 3)
        .reshape(128, NG * NCHUNK * GW)
    )
    maskA = (np.ones if h == 0 else np.zeros)((128, 128), bf)
    kk = np.arange(128)
    maskB = np.where(kk[:, None] <= kk[None, :], 1.0, 0.0).astype(bf)
    # cst32: [bq, bk*SCALE, pad, bvv(128 cols, bv broadcast to all parts)]
    cst32 = np.zeros((128, 3 + DK), np.float32)
    cst32[:, 0] = np.asarray(bq, np.float32)
    cst32[:, 1] = np.asarray(bk, np.float32) * SCALE
    cst32[:, 3:] = np.asarray(bv, np.float32)[None, :]
    cstf8 = np.concatenate([_pack_w_fp8(Wk), _pack_w_fp8(Wq)], axis=1)
    cst16 = np.concatenate(
        [_pack_w_bf16(Wv), maskA, maskB], axis=1
    )
    return {
        "x8p": x8p,
        "xgp": xgp,
        "cstf8": np.ascontiguousarray(cstf8),
        "cst32": np.ascontiguousarray(cst32),
        "cst16": np.ascontiguousarray(cst16),
    }


def gather_out(results):
    """results: list of 8 dicts with 'o' [1024, 128] -> full [B, T, DK]."""
    out = np.zeros((B, T, DK), np.float32)
    for core in range(8):
        b, h = core // 2, core % 2
        ob = results[core]["o"]
        for p in range(NSLOT):
            g = 2 * p + 1 - h
            out[b, 128 * g : 128 * (g + 1), :] = ob[128 * p : 128 * (p + 1), :]
    return out


def kernel(x, Wq, bq, Wk, bk, Wv, bv):
    import ml_dtypes
    from concourse.bass_utils import run_bass_kernel_spmd

    x = np.asarray(x, np.float32)
    args = [np.asarray(a, np.float32) for a in (Wq, bq, Wk, bk, Wv, bv)]
    nc = get_built()
    # one transpose+cast per batch, shared by its two wedge cores
    xT_pres = [np.ascontiguousarray(x[b].T.astype(ml_dtypes.bfloat16))
               for b in range(B)]
    x8T_pres = [np.ascontiguousarray(x[b].T.astype(ml_dtypes.float8_e4m3))
                for b in range(B)]
    in_maps = [
        make_in_map(x[core // 2], args[0], args[1], args[2], args[3], args[4],
                    args[5], core % 2, xT_pre=xT_pres[core // 2],
                    x8T_pre=x8T_pres[core // 2])
        for core in range(8)
    ]
    res = run_bass_kernel_spmd(nc, in_maps, core_ids=list(range(8)))
    return gather_out(res.results)


if __name__ == "__main__":
    rng = np.random.default_rng(0)
    x = rng.standard_normal((B, T, D), dtype=np.float32)
    Wq = rng.standard_normal((D, DK), dtype=np.float32) * 0.03
    out = kernel(x, Wq, np.zeros(DK, np.float32), Wq, np.zeros(DK, np.float32),
                 Wq, np.zeros(DK, np.float32))
    print(out.shape)
